# revision 19
# baseline (speedup 1.0000x reference)
"""AttentionUserEmbedding Trainium2 Bass kernel.

Problem: out[b, :] = sum_l softmax_l(mask(x[b] @ w))[l] * x[b, l, :]
  x: [8192, 200, 64] f32, lengths: [8192] i64 (ragged mask), w: [64] f32.

Sharding: pure data parallel over batch; 1024 batches per core on 8 cores.
Per-core layout: tiles of P=128 batches on the SBUF partition dim,
free dim = L*D = 12800 f32 (50KB/partition/tile).

Raw-bass implementation (explicit semaphores; the local walrus build
allows only one embedded sync wait per instruction, so all multi-proc
synchronization is done with standalone wait_ge instructions).

Per-tile compute:
  scores  = reduce_d(x * w)                  (DVE)
  mask    = arange < len; masked = select    (DVE)
  -max    = reduce_max(negate)               (DVE)
  e       = exp(masked - max), sumexp        (Act, fused accumulate)
  r       = 1/sumexp                         (DVE)
  ex      = x * e   (split DVE | Pool)       (e broadcast over d)
  acc     = reduce_l(ex)  (strided view)     (DVE)
  out     = acc * r                          (DVE)
"""

from contextlib import ExitStack

import numpy as np

import concourse.bass as bass
from concourse import mybir
from concourse.bass_utils import run_bass_kernel_spmd

B, L, D = 8192, 200, 64
N_CORES = 8
B_SHARD = B // N_CORES  # 1024
P = 128  # batches per tile (SBUF partition dim)
LD = L * D

F32 = mybir.dt.float32


def _ap(tensor, offset, dims):
    return bass.AP(tensor=tensor, offset=offset, ap=[list(d) for d in dims])


def _view3(handle):
    """[P, LD] sbuf tensor viewed as [P, L, D]."""
    a = handle.ap()
    return _ap(a.tensor, a.offset, [a.ap[0], [D, L], [1, D]])


def _ebcast(handle, l0, l1):
    """e[P, L] sbuf tensor viewed as [P, l1-l0, D] with d stride 0."""
    a = handle.ap()
    return _ap(a.tensor, a.offset + l0, [a.ap[0], [1, l1 - l0], [0, D]])


def _attention_raw_v1(nc, x, lens, cvec, out, b_shard, d_split=200):
    ntiles = b_shard // P
    with ExitStack() as ctx:
        sb = lambda name, shape, dt=F32: ctx.enter_context(
            nc.sbuf_tensor(name, shape, dt)
        )
        sem = lambda name: ctx.enter_context(nc.semaphore(name))

        xt = [sb(f"xt{i}", [P, LD]) for i in range(2)]
        exb = sb("exb", [P, LD])
        cw = sb("cw", [P, D + L + 1])
        lenall = sb("lenall", [P, ntiles])
        scores = sb("scores", [P, L])
        mask = sb("mask", [P, L], mybir.dt.uint8)
        masked = sb("masked", [P, L])
        e = sb("e", [P, L])
        negmax = sb("negmax", [P, 1])
        sumexp = sb("sumexp", [P, 1])
        rinv = sb("rinv", [P, 1])
        acc = sb("acc", [P, D])
        outt = sb("outt", [P, 2 * D])

        q_x = [sem("q_x0"), sem("q_x1")]
        q_c = sem("q_c")
        q_o = [sem("q_o0"), sem("q_o1")]
        s_sm = sem("s_sm")
        s_exp = sem("s_exp")
        s_E = sem("s_E")
        s_pD = sem("s_pD")
        s_xf_d = sem("s_xf_d")
        s_xf_p = sem("s_xf_p")
        s_out = sem("s_out")

        cw_a = cw.ap()
        arange_ap = _ap(cw_a.tensor, cw_a.offset + D, [cw_a.ap[0], [1, L]])
        neginf_ap = _ap(cw_a.tensor, cw_a.offset + D + L, [cw_a.ap[0], [0, L]])
        w_bc = _ap(cw_a.tensor, cw_a.offset, [cw_a.ap[0], [0, L], [1, D]])
        len_a = lenall.ap()

        x_a = x.ap()
        out_a = out.ap()
        cvec_a = cvec.ap()
        lens_a = lens.ap()

        use_pool = d_split < L

        with nc.Block() as block:

            @block.sync
            def _(sp):
                sp.dma_start(
                    out=cw.ap(), in_=_ap(cvec_a.tensor, 0, [[0, P], [1, D + L + 1]])
                ).then_inc(q_c, 16)
                with nc.allow_non_contiguous_dma(reason="tiny lens load"):
                    sp.dma_start(
                        out=lenall.ap(),
                        in_=_ap(lens_a.tensor, 0, [[1, P], [P, ntiles]]),
                    ).then_inc(q_c, 16)
                for t in range(min(2, ntiles)):
                    sp.dma_start(
                        out=xt[t].ap(),
                        in_=_ap(x_a.tensor, t * P * LD, [[LD, P], [1, LD]]),
                    ).then_inc(q_x[t % 2], 16)
                for t in range(ntiles):
                    if t + 2 < ntiles:
                        sp.wait_ge(s_xf_d, t + 1)
                        if use_pool:
                            sp.wait_ge(s_xf_p, t + 1)
                        sp.dma_start(
                            out=xt[t % 2].ap(),
                            in_=_ap(
                                x_a.tensor, (t + 2) * P * LD, [[LD, P], [1, LD]]
                            ),
                        ).then_inc(q_x[t % 2], 16)
                    sp.wait_ge(s_out, t + 1)
                    o_a = outt.ap()
                    sp.dma_start(
                        out=_ap(out_a.tensor, t * P * D, [[D, P], [1, D]]),
                        in_=_ap(o_a.tensor, o_a.offset + (t % 2) * D, [o_a.ap[0], [1, D]]),
                    ).then_inc(q_o[t % 2], 16)
                for s in range(2):
                    n_s = (ntiles - s + 1) // 2
                    if n_s:
                        sp.wait_ge(q_o[s], 16 * n_s)

            @block.vector
            def _(v):
                v.wait_ge(q_c, 32)
                for t in range(ntiles):
                    v.wait_ge(q_x[t % 2], 16 * (t // 2 + 1))
                    x3 = _view3(xt[t % 2])
                    ex3 = _view3(exb)
                    # xw = x * w (reuses exb)
                    v.tensor_mul(ex3, x3, w_bc)
                    v.drain()  # same-engine RAW: exb writes must land
                    v.reduce_sum(scores.ap(), ex3, axis=mybir.AxisListType.X)
                    v.tensor_scalar(
                        out=mask.ap(),
                        in0=arange_ap,
                        scalar1=_ap(
                            len_a.tensor, len_a.offset + t, [len_a.ap[0], [1, 1]]
                        ),
                        scalar2=None,
                        op0=mybir.AluOpType.is_lt,
                    )
                    v.drain()  # scores + mask writes
                    v.select(
                        out=masked.ap(),
                        mask=mask.ap(),
                        on_true=scores.ap(),
                        on_false=neginf_ap,
                        add_drain=True,
                    )
                    v.drain()  # masked writes
                    v.tensor_reduce(
                        out=negmax.ap(),
                        in_=masked.ap(),
                        axis=mybir.AxisListType.X,
                        op=mybir.AluOpType.max,
                        negate=True,
                    ).then_inc(s_sm, 1)
                    v.wait_ge(s_exp, t + 1)
                    v.reciprocal(rinv.ap(), sumexp.ap())
                    # ex = x * e on l in [0, d_split)
                    v.tensor_mul(
                        _ap(ex3.tensor, ex3.offset, [ex3.ap[0], [D, d_split], [1, D]]),
                        _ap(x3.tensor, x3.offset, [x3.ap[0], [D, d_split], [1, D]]),
                        _ebcast(e, 0, d_split),
                    ).then_inc(s_xf_d, 1)
                    if use_pool:
                        v.wait_ge(s_pD, t + 1)
                    v.drain()  # ex + rinv writes
                    # acc[p, d] = sum_l ex  ([p, d, l] view, reduce innermost)
                    v.reduce_sum(
                        acc.ap(),
                        _ap(ex3.tensor, ex3.offset, [ex3.ap[0], [1, D], [D, L]]),
                        axis=mybir.AxisListType.X,
                    ).then_inc(s_E, 1)
                    if t >= 2:
                        v.wait_ge(q_o[t % 2], 16 * (t // 2))
                    v.drain()  # acc writes
                    o_a = outt.ap()
                    v.tensor_scalar_mul(
                        _ap(o_a.tensor, o_a.offset + (t % 2) * D, [o_a.ap[0], [1, D]]),
                        acc.ap(),
                        rinv.ap(),
                    ).then_inc(s_out, 1)

            @block.scalar
            def _(a):
                for t in range(ntiles):
                    a.wait_ge(s_sm, t + 1)
                    a.activation(
                        out=e.ap(),
                        in_=masked.ap(),
                        func=mybir.ActivationFunctionType.Exp,
                        bias=negmax.ap(),
                        scale=1.0,
                        accum_out=sumexp.ap(),
                    ).then_inc(s_exp, 1)

            if use_pool:

                @block.gpsimd
                def _(p):
                    for t in range(ntiles):
                        p.wait_ge(s_exp, t + 1)
                        if t > 0:
                            p.wait_ge(s_E, t)
                        x3 = _view3(xt[t % 2])
                        ex3 = _view3(exb)
                        n_l = L - d_split
                        p.tensor_mul(
                            _ap(
                                ex3.tensor,
                                ex3.offset + d_split * D,
                                [ex3.ap[0], [D, n_l], [1, D]],
                            ),
                            _ap(
                                x3.tensor,
                                x3.offset + d_split * D,
                                [x3.ap[0], [D, n_l], [1, D]],
                            ),
                            _ebcast(e, d_split, L),
                        ).then_inc(s_pD, 1)
                        p.sem_inc(s_xf_p, 1)


def _attention_raw_v2(nc, x, lens, cvec, ident, wpe, out, b_shard, d_split=64):
    """PE computes scores: per 128-col chunk of x, PE-transposes into PSUM,
    Act copies to SBUF, then a [128,2] fp32r matmul against block-diagonal w
    writes scores[b, 2c:2c+2] into a PSUM accumulator. The weighted-sum
    multiply splits between DVE ([0, d_split)) and Pool ([d_split, L))."""
    ntiles = b_shard // P
    F32R = mybir.dt.float32r
    CHUNKS = LD // P  # 100
    G = 8  # chunks per group
    NG = (CHUNKS + G - 1) // G  # 13 groups (12x8 + 1x4)

    def nch(g):
        return G if g < NG - 1 else CHUNKS - (NG - 1) * G

    with ExitStack() as ctx:
        sb = lambda name, shape, dt=F32: ctx.enter_context(
            nc.sbuf_tensor(name, shape, dt)
        )
        ps = lambda name, shape: ctx.enter_context(nc.psum_tensor(name, shape, F32))
        sem = lambda name: ctx.enter_context(nc.semaphore(name))

        xt = [sb(f"xt{i}", [P, LD]) for i in range(2)]
        exb = sb("exb", [P, LD])
        cw = sb("cw", [P, D + L + 1])
        lenall = sb("lenall", [P, ntiles])
        ident_sb = sb("identsb", [P, P])
        wpe_sb = sb("wpesb", [P, 2])
        sbT = [sb(f"sbT{i}", [P, G * P]) for i in range(3)]
        mask = sb("mask", [P, L], mybir.dt.uint8)
        masked = sb("masked", [P, L])
        e = sb("e", [P, L])
        negmax = sb("negmax", [P, 1])
        sumexp = sb("sumexp", [P, 1])
        rinv = sb("rinv", [P, 1])
        acc = sb("acc", [P, D])
        outt = sb("outt", [P, 2 * D])

        psT = [ps(f"psT{i}", [P, G * P]) for i in range(2)]
        scps = [ps(f"scps{i}", [P, L]) for i in range(2)]

        q_x = [sem("q_x0"), sem("q_x1")]
        q_c = sem("q_c")
        q_o = [sem("q_o0"), sem("q_o1")]
        s_tr = sem("s_tr")
        s_cp = sem("s_cp")
        s_mm = sem("s_mm")
        s_sel = sem("s_sel")
        s_sm = sem("s_sm")
        s_exp = sem("s_exp")
        s_E = sem("s_E")
        s_pD = sem("s_pD")
        s_xf_d = sem("s_xf_d")
        s_out = sem("s_out")

        cw_a = cw.ap()
        arange_ap = _ap(cw_a.tensor, cw_a.offset + D, [cw_a.ap[0], [1, L]])
        neginf_ap = _ap(cw_a.tensor, cw_a.offset + D + L, [cw_a.ap[0], [0, L]])
        len_a = lenall.ap()
        x_a = x.ap()
        out_a = out.ap()
        cvec_a = cvec.ap()
        lens_a = lens.ap()
        ident_r = ident_sb.ap().bitcast(F32R)
        wpe_r = wpe_sb.ap().bitcast(F32R)

        with nc.Block() as block:

            @block.sync
            def _(sp):
                sp.dma_start(
                    out=cw.ap(), in_=_ap(cvec_a.tensor, 0, [[0, P], [1, D + L + 1]])
                ).then_inc(q_c, 16)
                with nc.allow_non_contiguous_dma(reason="tiny lens load"):
                    sp.dma_start(
                        out=lenall.ap(),
                        in_=_ap(lens_a.tensor, 0, [[1, P], [P, ntiles]]),
                    ).then_inc(q_c, 16)
                sp.dma_start(out=ident_sb.ap(), in_=ident.ap()).then_inc(q_c, 16)
                sp.dma_start(out=wpe_sb.ap(), in_=wpe.ap()).then_inc(q_c, 16)
                for t in range(min(2, ntiles)):
                    sp.dma_start(
                        out=xt[t].ap(),
                        in_=_ap(x_a.tensor, t * P * LD, [[LD, P], [1, LD]]),
                    ).then_inc(q_x[t % 2], 16)
                for t in range(ntiles):
                    if t + 2 < ntiles:
                        sp.wait_ge(s_xf_d, t + 1)
                        sp.wait_ge(s_pD, t + 1)
                        sp.wait_ge(s_tr, NG * (t + 1))
                        sp.dma_start(
                            out=xt[t % 2].ap(),
                            in_=_ap(
                                x_a.tensor, (t + 2) * P * LD, [[LD, P], [1, LD]]
                            ),
                        ).then_inc(q_x[t % 2], 16)
                    sp.wait_ge(s_out, t + 1)
                    o_a = outt.ap()
                    sp.dma_start(
                        out=_ap(out_a.tensor, t * P * D, [[D, P], [1, D]]),
                        in_=_ap(
                            o_a.tensor, o_a.offset + (t % 2) * D, [o_a.ap[0], [1, D]]
                        ),
                    ).then_inc(q_o[t % 2], 16)
                for s in range(2):
                    n_s = (ntiles - s + 1) // 2
                    if n_s:
                        sp.wait_ge(q_o[s], 16 * n_s)

            @block.tensor
            def _(pe):
                pe.wait_ge(q_c, 64)

                def emit_matmuls(t, g):
                    gi = t * NG + g
                    pe.wait_ge(s_cp, gi + 1)
                    sa = sbT[gi % 3].ap()
                    sc_a = scps[t % 2].ap()
                    last = None
                    for i in range(nch(g)):
                        c = g * G + i
                        last = pe.matmul(
                            out=_ap(
                                sc_a.tensor,
                                sc_a.offset + 2 * c,
                                [sc_a.ap[0], [1, 2]],
                            ),
                            lhsT=_ap(
                                sa.tensor, sa.offset + i * P, [sa.ap[0], [1, P]]
                            ).bitcast(F32R),
                            rhs=wpe_r,
                            start=True,
                            stop=True,
                        )
                    last.then_inc(s_mm, 1)

                for t in range(ntiles):
                    pe.wait_ge(q_x[t % 2], 16 * (t // 2 + 1))
                    if t >= 2:
                        pe.wait_ge(s_sel, t - 1)
                    xt_a = xt[t % 2].ap()
                    for g in range(NG):
                        gi = t * NG + g
                        if gi >= 2:
                            pe.wait_ge(s_cp, gi - 1)
                        pa = psT[gi % 2].ap()
                        last = None
                        for i in range(nch(g)):
                            c = g * G + i
                            last = pe.transpose(
                                out=_ap(
                                    pa.tensor, pa.offset + i * P, [pa.ap[0], [1, P]]
                                ).bitcast(F32R),
                                in_=_ap(
                                    xt_a.tensor,
                                    xt_a.offset + c * P,
                                    [xt_a.ap[0], [1, P]],
                                ).bitcast(F32R),
                                identity=ident_r,
                            )
                        last.then_inc(s_tr, 1)
                        if g >= 1:
                            emit_matmuls(t, g - 1)
                    emit_matmuls(t, NG - 1)

            @block.scalar
            def _(a):
                for t in range(ntiles):
                    for g in range(NG):
                        gi = t * NG + g
                        a.wait_ge(s_tr, gi + 1)
                        if gi >= 3:
                            a.wait_ge(s_mm, gi - 2)
                        n = nch(g) * P
                        pa = psT[gi % 2].ap()
                        sa = sbT[gi % 3].ap()
                        a.copy(
                            out=_ap(sa.tensor, sa.offset, [sa.ap[0], [1, n]]),
                            in_=_ap(pa.tensor, pa.offset, [pa.ap[0], [1, n]]),
                        ).then_inc(s_cp, 1)
                    a.wait_ge(s_sm, t + 1)
                    a.activation(
                        out=e.ap(),
                        in_=masked.ap(),
                        func=mybir.ActivationFunctionType.Exp,
                        bias=negmax.ap(),
                        scale=1.0,
                        accum_out=sumexp.ap(),
                    ).then_inc(s_exp, 1)

            @block.vector
            def _(v):
                v.wait_ge(q_c, 64)
                for t in range(ntiles):
                    x3 = _view3(xt[t % 2])
                    ex3 = _view3(exb)
                    v.tensor_scalar(
                        out=mask.ap(),
                        in0=arange_ap,
                        scalar1=_ap(
                            len_a.tensor, len_a.offset + t, [len_a.ap[0], [1, 1]]
                        ),
                        scalar2=None,
                        op0=mybir.AluOpType.is_lt,
                    )
                    v.wait_ge(s_mm, NG * (t + 1))
                    v.drain()  # mask writes
                    v.select(
                        out=masked.ap(),
                        mask=mask.ap(),
                        on_true=scps[t % 2].ap(),
                        on_false=neginf_ap,
                        add_drain=True,
                    ).then_inc(s_sel, 1)
                    v.drain()  # masked writes
                    v.tensor_reduce(
                        out=negmax.ap(),
                        in_=masked.ap(),
                        axis=mybir.AxisListType.X,
                        op=mybir.AluOpType.max,
                        negate=True,
                    ).then_inc(s_sm, 1)
                    v.wait_ge(s_exp, t + 1)
                    v.reciprocal(rinv.ap(), sumexp.ap())
                    v.tensor_mul(
                        _ap(ex3.tensor, ex3.offset, [ex3.ap[0], [D, d_split], [1, D]]),
                        _ap(x3.tensor, x3.offset, [x3.ap[0], [D, d_split], [1, D]]),
                        _ebcast(e, 0, d_split),
                    ).then_inc(s_xf_d, 1)
                    v.wait_ge(s_pD, t + 1)
                    v.drain()  # ex + rinv writes
                    v.reduce_sum(
                        acc.ap(),
                        _ap(ex3.tensor, ex3.offset, [ex3.ap[0], [1, D], [D, L]]),
                        axis=mybir.AxisListType.X,
                    ).then_inc(s_E, 1)
                    if t >= 2:
                        v.wait_ge(q_o[t % 2], 16 * (t // 2))
                    v.drain()  # acc writes
                    o_a = outt.ap()
                    v.tensor_scalar_mul(
                        _ap(o_a.tensor, o_a.offset + (t % 2) * D, [o_a.ap[0], [1, D]]),
                        acc.ap(),
                        rinv.ap(),
                    ).then_inc(s_out, 1)

            @block.gpsimd
            def _(p):
                for t in range(ntiles):
                    p.wait_ge(s_exp, t + 1)
                    if t > 0:
                        p.wait_ge(s_E, t)
                    x3 = _view3(xt[t % 2])
                    ex3 = _view3(exb)
                    n_l = L - d_split
                    p.tensor_mul(
                        _ap(
                            ex3.tensor,
                            ex3.offset + d_split * D,
                            [ex3.ap[0], [D, n_l], [1, D]],
                        ),
                        _ap(
                            x3.tensor,
                            x3.offset + d_split * D,
                            [x3.ap[0], [D, n_l], [1, D]],
                        ),
                        _ebcast(e, d_split, L),
                    ).then_inc(s_pD, 1)


def _make_ident():
    return np.eye(P, dtype=np.float32)


def _make_wpe(attn_w):
    w = np.asarray(attn_w, dtype=np.float32)
    W = np.zeros((P, 2), np.float32)
    W[:D, 0] = w
    W[D:, 1] = w
    return W


VARIANT = "v1"
D_SPLIT = {"v1": 200, "v2": 64}


def _build_program(b_shard=B_SHARD, variant=None, d_split=None):
    variant = variant or VARIANT
    if d_split is None:
        d_split = D_SPLIT[variant]
    nc = bass.Bass("TRN2", target_bir_lowering=False, debug=False)
    x = nc.dram_tensor("x", [b_shard, L, D], F32, kind="ExternalInput")
    lens = nc.dram_tensor("lens", [b_shard], F32, kind="ExternalInput")
    cvec = nc.dram_tensor("cvec", [D + L + 1], F32, kind="ExternalInput")
    if variant == "v2":
        ident = nc.dram_tensor("ident", [P, P], F32, kind="ExternalInput")
        wpe = nc.dram_tensor("wpe", [P, 2], F32, kind="ExternalInput")
    out = nc.dram_tensor("out", [b_shard, D], F32, kind="ExternalOutput")
    if variant == "v2":
        _attention_raw_v2(nc, x, lens, cvec, ident, wpe, out, b_shard, d_split=d_split)
    else:
        _attention_raw_v1(nc, x, lens, cvec, out, b_shard, d_split=d_split)
    return nc


_PROGRAMS = {}


def _get_program(b_shard):
    if b_shard not in _PROGRAMS:
        _PROGRAMS[b_shard] = _build_program(b_shard)
    return _PROGRAMS[b_shard]


def _make_cvec(attn_w):
    w = np.ascontiguousarray(np.asarray(attn_w, dtype=np.float32))
    return np.concatenate(
        [w, np.arange(L, dtype=np.float32), np.float32([-1.0e30])]
    ).astype(np.float32)


def _run(padded_embeddings, lengths, attn_w, trace=False, **spmd_kwargs):
    x = np.ascontiguousarray(np.asarray(padded_embeddings, dtype=np.float32))
    lens = np.asarray(lengths).astype(np.float32)
    b = x.shape[0]
    b_shard = b // N_CORES
    nc = _get_program(b_shard)
    cvec = _make_cvec(attn_w)
    in_maps = [
        {
            "x": np.ascontiguousarray(x[c * b_shard : (c + 1) * b_shard]),
            "lens": np.ascontiguousarray(lens[c * b_shard : (c + 1) * b_shard]),
            "cvec": cvec,
        }
        for c in range(N_CORES)
    ]
    res = run_bass_kernel_spmd(
        nc, in_maps, core_ids=list(range(N_CORES)), trace=trace, **spmd_kwargs
    )
    out = np.concatenate([r["out"] for r in res.results], axis=0)
    return out, res


def kernel(padded_embeddings, lengths, attn_w):
    out, _ = _run(padded_embeddings, lengths, attn_w)
    return out


def benchmark(padded_embeddings, lengths, attn_w, iters=30):
    """Time repeated NEFF executions with device-resident inputs.

    Mirrors bass2jax.run_bass_via_pjrt's shard_map construction but keeps
    inputs on device and re-executes, returning (output, per-iter ns).
    """
    import time

    import jax
    import concourse.mybir as mybir_
    from concourse import bass2jax
    from jax.sharding import Mesh, NamedSharding, PartitionSpec
    from jax.experimental.shard_map import shard_map

    bass2jax.install_neuronx_cc_hook()

    x = np.ascontiguousarray(np.asarray(padded_embeddings, dtype=np.float32))
    lens = np.asarray(lengths).astype(np.float32)
    b = x.shape[0]
    b_shard = b // N_CORES
    nc = _get_program(b_shard)
    cvec = _make_cvec(attn_w)

    partition_name = nc.partition_id_tensor.name if nc.partition_id_tensor else None
    in_names, out_names, out_avals, zero_outs = [], [], [], []
    for alloc in nc.m.functions[0].allocations:
        if not isinstance(alloc, mybir_.MemoryLocationSet):
            continue
        name = alloc.memorylocations[0].name
        if alloc.kind == "ExternalInput":
            if name != partition_name:
                in_names.append(name)
        elif alloc.kind == "ExternalOutput":
            out_names.append(name)
            shape = tuple(alloc.tensor_shape)
            dtype = mybir_.dt.np(alloc.dtype)
            out_avals.append(jax.core.ShapedArray(shape, dtype))
            zero_outs.append(np.zeros((N_CORES * shape[0], *shape[1:]), dtype))
    n_params = len(in_names)
    all_names = in_names + out_names
    if partition_name is not None:
        all_names = all_names + [partition_name]

    def _body(*args):
        operands = list(args)
        if partition_name is not None:
            operands.append(bass2jax.partition_id_tensor())
        outs = bass2jax._bass_exec_p.bind(
            *operands,
            out_avals=tuple(out_avals),
            in_names=tuple(all_names),
            out_names=tuple(out_names),
            lowering_input_output_aliases=(),
            sim_require_finite=True,
            sim_require_nnan=True,
            nc=nc,
        )
        return tuple(outs)

    devices = jax.devices()[:N_CORES]
    mesh = Mesh(np.asarray(devices), ("core",))
    n_outs = len(out_names)
    fn = jax.jit(
        shard_map(
            _body,
            mesh=mesh,
            in_specs=(PartitionSpec("core"),) * (n_params + n_outs),
            out_specs=(PartitionSpec("core"),) * n_outs,
            check_rep=False,
        ),
        keep_unused=True,
    )

    host_ins = {
        "x": x,
        "lens": lens,
        "cvec": np.concatenate([cvec] * N_CORES, axis=0),
    }
    sh = NamedSharding(mesh, PartitionSpec("core"))
    dev_args = [jax.device_put(host_ins[n], sh) for n in in_names]
    dev_zeros = [jax.device_put(z, sh) for z in zero_outs]

    outs = fn(*dev_args, *dev_zeros)
    jax.block_until_ready(outs)
    times = []
    for _ in range(iters):
        t0 = time.perf_counter()
        outs = fn(*dev_args, *dev_zeros)
        jax.block_until_ready(outs)
        times.append((time.perf_counter() - t0) * 1e9)

    out_full = np.asarray(outs[out_names.index("out")])
    return out_full, times
